# revision 14
# baseline (speedup 1.0000x reference)
"""Multi-head attention (B=2, T=2048, D=1024, H=16, no causal mask) on 8 trn2
NeuronCores — head-parallel sharding with an all-gather exchange before o_proj.

Sharding: core c -> batch b = c//4, head-group g = c%4 (heads [4g, 4g+4), i.e.
local pairs p=0,1 = global pairs 2g+p).  Each core computes Q/K/V projections
and attention for its 4 heads over ALL 2048 queries of its batch (8.6 GFLOP
vs 15.1 for the old data-parallel scheme), then the cores of a batch exchange
attention outputs and each core runs o_proj for its 512-query slice
qs = [g*512, (g+1)*512).

Exchange: 8 small 4-core AllGathers (one per (pair, q-block)) fired as each
(pair, qb) block finishes normalize, so all but the last overlap attention.
AllToAll is unsupported for 4-core groups, so each AG delivers all 4 cores'
[128, 512] blocks and the consumer selects its own q-block with a per-core
one-hot mask input (SPMD-safe: rank enters via input data only) using fused
DVE multiply-adds.

Per-core pipeline (fp16 compute, fp32 PSUM):
  1. PE-transpose X[b] -> XT [128, kd*2048].
  2. QT/KT per pair: [128 (2 heads x 64), 2048]; V slots [128, kc*(4*65)]
     (65-wide: ones column makes PV also produce the softmax denominator).
  3. Attention pair-outer, qb-inner, kc innermost:
       logitsT via two row-tiled (tile_position (0,0)/(64,0)) K=64 matmuls
       that run CONCURRENTLY on the PE -> lg psum [128, 1024]
       exp on ScalarE (the only ACT-engine work in the kernel; ~138 us total,
       near-critical) -> PT f16
       PV accumulate [65, 512] psum over kc.
     normalize: 1/s via reciprocal_approx_fast, DMA partition-broadcast, DVE
     muls; head b partition-shifted via SBUF DMA.
  4. o_proj split: pair-0 slots accumulated into y_acc (SBUF f32) while
     pair-1 attention runs; pair-1 slots added at the tail.
Leftover projection work (V chunks, QT/KT pair 1) is emitted as PE filler
inside the attention loops (the attention inner loop alone is ACT-bound).
"""

import numpy as np

import concourse.bacc as bacc
import concourse.mybir as mybir
import concourse.tile as tile
from concourse.masks import make_identity

F32 = mybir.dt.float32
F16 = mybir.dt.float16

B, T, D, H = 2, 2048, 1024, 16
DH = D // H          # 64
P = 128
KD = D // P          # 8 contraction chunks over D
NT = T // P          # 16 key-token chunks
TQ = 512             # q-block size == o_proj q-slice
NQB = T // TQ        # 4 q blocks
NG = 4               # head-groups (cores per batch)
HPG = H // NG        # 4 heads per core
NPL = HPG // 2       # 2 local pairs
VW = DH + 1          # 65: V head slot width incl. ones column
NQC = TQ // P        # 4 128-row chunks per q-slice
N_CORES = 8
EXPF = mybir.ActivationFunctionType.Exp
MUL = mybir.AluOpType.mult
ADD = mybir.AluOpType.add

RG = [[0, 1, 2, 3], [4, 5, 6, 7]]


DEBUG_DUMPS = False
PRE_PROJ_P1 = False


def build_nc():
    nc = bacc.Bacc("TRN2", target_bir_lowering=False, debug=False,
                   num_devices=N_CORES)
    xb = nc.dram_tensor("xb", [T, D], F16, kind="ExternalInput").ap()
    wq = nc.dram_tensor("wq", [D, HPG * DH], F16, kind="ExternalInput").ap()
    wk = nc.dram_tensor("wk", [D, HPG * DH], F16, kind="ExternalInput").ap()
    wv = nc.dram_tensor("wv", [D, HPG * DH], F16, kind="ExternalInput").ap()
    wo = nc.dram_tensor("wo", [D, D], F16, kind="ExternalInput").ap()
    msk = nc.dram_tensor("msk", [P, NQB], F32, kind="ExternalInput").ap()
    y = nc.dram_tensor("y", [TQ, D], F32, kind="ExternalOutput").ap()
    if DEBUG_DUMPS:
        dbg_ot = nc.dram_tensor("dbg_ot", [P, NPL * T], F16,
                                kind="ExternalOutput").ap()
        dbg_sel = nc.dram_tensor("dbg_sel", [P, H // 2 * TQ], F16,
                                 kind="ExternalOutput").ap()
        dbg_qt = nc.dram_tensor("dbg_qt", [P, NPL * T], F16,
                                kind="ExternalOutput").ap()
        dbg_kt = nc.dram_tensor("dbg_kt", [P, NPL * T], F16,
                                kind="ExternalOutput").ap()

    with tile.TileContext(nc) as tc:
      with (
          tc.tile_pool(name="persist", bufs=1) as persist,
          tc.tile_pool(name="auxps", bufs=2, space="PSUM") as auxps,
          tc.tile_pool(name="xin", bufs=4) as xinp,
          tc.tile_pool(name="ptp", bufs=4) as ptp,
          tc.tile_pool(name="rbp", bufs=4) as rbp,
          tc.tile_pool(name="ginp", bufs=8) as ginp,
          tc.tile_pool(name="dram", bufs=1, space="DRAM") as dram,
      ):
        xt = persist.tile([P, KD * T], F16)          # 32 KB/part
        qt = [persist.tile([P, T], F16, name=f"qt{i}") for i in range(NPL)]
        kt = [persist.tile([P, T], F16, name=f"kt{i}") for i in range(NPL)]
        v_sb = persist.tile([P, NT * HPG * VW], F16)  # 8.1 KB
        ot = [persist.tile([P, T], F16, name=f"ot{i}") for i in range(NPL)]
        ot_sel = persist.tile([P, H // 2 * TQ], F16)  # 8 KB (8 slots x 512)
        wo_sb = persist.tile([P, (H // 2) * D], F16)  # 16 KB [p, slot, 1024]
        y_acc = persist.tile([P, NQC * D], F32)      # 16 KB
        msk_sb = persist.tile([P, NQB], F32)
        ident = persist.tile([P, P], F16)
        make_identity(nc, ident)
        # ones columns in every (tok-chunk, head) V slot
        onec = persist.tile([P, 1], F16)
        nc.vector.memset(onec[:], 1.0)
        nc.vector.tensor_copy(
            v_sb.rearrange("p (b c) -> p b c", c=VW)[:, :, DH:DH + 1],
            onec.unsqueeze(1).broadcast_to((P, NT * HPG, 1)))

        # weight slices viewed [p, kd, cols] for stationary use
        wq_sb = persist.tile([P, KD * HPG * DH], F16, name="wq_sb")  # 4 KB
        wk_sb = persist.tile([P, KD * HPG * DH], F16, name="wk_sb")
        wv_sb = persist.tile([P, KD * HPG * DH], F16, name="wv_sb")

        def load_small_weights():
            # emitted after the first x chunk so the x DMAs lead the queues
            for w_sb, w in ((wq_sb, wq), (wk_sb, wk), (wv_sb, wv)):
                nc.sync.dma_start(
                    w_sb.rearrange("p (kd c) -> p kd c", c=HPG * DH),
                    w.rearrange("(kd p) c -> p kd c", p=P))
            nc.sync.dma_start(msk_sb[:], msk)

        def load_wo():
            # wo only feeds o_proj; keep its 2 MB strided DMA off the
            # critical input path
            nc.sync.dma_start(
                wo_sb.rearrange("p (s c) -> p s c", c=D),
                wo.rearrange("(s p) c -> p s c", p=P))

        ag_in = {}
        ag_out = {}
        for pl in range(NPL):
            for qb in range(NQB):
                ag_in[pl, qb] = dram.tile([P, TQ], F16,
                                          name=f"agin_{pl}_{qb}")
                ag_out[pl, qb] = dram.tile([NG, P, TQ], F16,
                                           name=f"agout_{pl}_{qb}")

        # ---------- helpers ------------------------------------------
        def proj_qk_block(w_sb, dst, pl, qb):
            # dst[pl][:, qb*TQ :] = W[:, pair pl]^T @ XT[:, qb block]
            pq = auxps.tile([P, TQ], F32, tag="aux")
            for kd in range(KD):
                nc.tensor.matmul(
                    pq[:],
                    w_sb[:, kd * HPG * DH + pl * P:
                         kd * HPG * DH + (pl + 1) * P],
                    xt[:, kd * T + qb * TQ: kd * T + (qb + 1) * TQ],
                    start=(kd == 0), stop=(kd == KD - 1))
            nc.vector.tensor_copy(
                dst[pl][:, qb * TQ: (qb + 1) * TQ], pq[:])

        def proj_v_chunk(tci):
            # V rows for tokens [tci*128, ..): [128, 256] -> 65-wide slots
            pv = auxps.tile([P, TQ], F32, tag="aux")
            for kd in range(KD):
                nc.tensor.matmul(
                    pv[:, 0:HPG * DH],
                    xt[:, kd * T + tci * P: kd * T + (tci + 1) * P],
                    wv_sb[:, kd * HPG * DH:(kd + 1) * HPG * DH],
                    start=(kd == 0), stop=(kd == KD - 1))
            dst = v_sb[:, tci * (HPG * VW): (tci + 1) * (HPG * VW)]
            nc.vector.tensor_copy(
                dst.rearrange("p (h c) -> p h c", c=VW)[:, :, 0:DH],
                pv[:, 0:HPG * DH].rearrange("p (h c) -> p h c", c=DH))

        # select-accumulate AG output block into ot_sel slots
        def gather_select(pl, qb):
            for r in range(NG):
                gin = ginp.tile([P, TQ], F16, tag="gin")
                nc.sync.dma_start(gin[:], ag_out[pl, qb][r])
                slot = 2 * r + pl
                dst = ot_sel[:, slot * TQ:(slot + 1) * TQ]
                if qb == 0:
                    nc.vector.tensor_scalar_mul(
                        dst, gin[:], msk_sb[:, qb:qb + 1])
                else:
                    nc.vector.scalar_tensor_tensor(
                        dst, gin[:], msk_sb[:, qb:qb + 1], dst,
                        op0=MUL, op1=ADD)

        def oproj_half(pl):
            # accumulate this pair-half's 4 slots into y_acc / emit y
            for qc in range(NQC):
                for nh in range(2):
                    py = auxps.tile([P, TQ], F32, tag="aux")
                    for r in range(NG):
                        slot = 2 * r + pl
                        nc.tensor.matmul(
                            py[:],
                            ot_sel[:, slot * TQ + qc * P:
                                   slot * TQ + (qc + 1) * P],
                            wo_sb[:, slot * D + nh * TQ:
                                  slot * D + (nh + 1) * TQ],
                            start=(r == 0), stop=(r == NG - 1))
                    ya = y_acc[:, qc * D + nh * TQ: qc * D + (nh + 1) * TQ]
                    if pl == 0:
                        nc.vector.tensor_copy(ya, py[:])
                    else:
                        yo = rbp.tile([P, TQ], F32, tag="yout", bufs=3)
                        nc.vector.tensor_add(yo[:], py[:], ya)
                        nc.sync.dma_start(
                            y[qc * P:(qc + 1) * P, nh * TQ:(nh + 1) * TQ],
                            yo[:])

        # ---------- pre-region ---------------------------------------
        with tc.tile_pool(name="trps", bufs=3, space="PSUM") as trps:
            for tci in range(NT):
                xin = xinp.tile([P, D], F16, tag="xin")
                nc.sync.dma_start(xin[:], xb[tci * P:(tci + 1) * P, :])
                if tci == 0:
                    load_small_weights()
                if tci == NT - 1:
                    load_wo()
                ps = trps.tile([P, KD * P], F16, tag="tr")
                for kd in range(KD):
                    nc.tensor.transpose(
                        ps[:, kd * P:(kd + 1) * P],
                        xin[:, kd * P:(kd + 1) * P], ident[:])
                nc.vector.tensor_copy(
                    xt.rearrange("p (k c) -> p k c", c=T)
                      [:, :, tci * P:(tci + 1) * P],
                    ps.rearrange("p (k c) -> p k c", c=P))

        for qb in range(NQB):
            proj_qk_block(wk_sb, kt, 0, qb)
        proj_qk_block(wq_sb, qt, 0, 0)
        for tci in range(2):
            proj_v_chunk(tci)

        # filler work emitted inside the attention loops, keyed by
        # (pair, qb, kc) slot.  Each item is a closure.
        filler = {}

        def add_filler(pl, qb, kc, fn):
            filler.setdefault((pl, qb, kc), []).append(fn)

        # V chunks tci=2..15 inside pair0 qb0: PV of kc reads v chunk kc,
        # so chunk tci must be emitted before PV tci — keep 2 ahead.
        for tci in range(2, NT):
            add_filler(0, 0, tci - 2, lambda t=tci: proj_v_chunk(t))
        # QT pair0 qb+1 emitted mid-way through qb
        for qb in range(NQB - 1):
            add_filler(0, qb, 10, lambda q=qb + 1:
                       proj_qk_block(wq_sb, qt, 0, q))
        # KT/QT pair1 during pair0 qb2/qb3
        for qb in range(NQB):
            add_filler(0, 2, 1 + 4 * qb, lambda q=qb:
                       proj_qk_block(wk_sb, kt, 1, q))
            add_filler(0, 3, 1 + 4 * qb, lambda q=qb:
                       proj_qk_block(wq_sb, qt, 1, q))
        if PRE_PROJ_P1:
            # debug/workaround: run pair-1 projections sequentially in the
            # pre-region instead of interleaved with attention.
            for key in [k for k in filler if k[0] == 0 and k[1] >= 2]:
                for fn in filler.pop(key):
                    fn()
        # pair0 gathers have landed by mid-pair1; fold their o_proj
        # half in as PE filler.
        add_filler(1, 1, 8, lambda: oproj_half(0))

        # ---------- attention ----------------------------------------
        with (
            tc.tile_pool(name="lgps", bufs=2, space="PSUM") as lgps,
            tc.tile_pool(name="pvps", bufs=2, space="PSUM") as pvps,
        ):
            for pl in range(NPL):
                for qb in range(NQB):
                    pva = pvps.tile([VW, TQ], F32, tag="pv")
                    pvb = pvps.tile([VW, TQ], F32, tag="pv")
                    for kc in range(NT):
                        for fn in filler.pop((pl, qb, kc), ()):
                            fn()
                        lg = lgps.tile([P, 2 * TQ], F32, tag="lg")
                        for hh in range(2):
                            nc.tensor.matmul(
                                lg[:, hh * TQ:(hh + 1) * TQ],
                                kt[pl][hh * DH:(hh + 1) * DH,
                                       kc * P:(kc + 1) * P],
                                qt[pl][hh * DH:(hh + 1) * DH,
                                       qb * TQ:(qb + 1) * TQ],
                                start=True, stop=True,
                                tile_position=(hh * DH, 0))
                        pt = ptp.tile([P, 2 * TQ], F16, tag="pt")
                        nc.scalar.activation(pt[:], lg[:], EXPF, scale=0.125)
                        for hh, pv_ in ((0, pva), (1, pvb)):
                            h = 2 * pl + hh
                            va = v_sb[:, kc * (HPG * VW) + h * VW:
                                      kc * (HPG * VW) + (h + 1) * VW]
                            nc.tensor.matmul(
                                pv_[:], va, pt[:, hh * TQ:(hh + 1) * TQ],
                                start=(kc == 0), stop=(kc == NT - 1))
                    # ---- normalize: ot[:, blk] = pv[0:64] / s ------------
                    srow = rbp.tile([1, 2 * TQ], F32, tag="srow")
                    nc.vector.tensor_copy(srow[0:1, 0:TQ],
                                          pva[DH:DH + 1, :])
                    nc.vector.tensor_copy(srow[0:1, TQ:2 * TQ],
                                          pvb[DH:DH + 1, :])
                    rc = rbp.tile([1, 2 * TQ], F32, tag="rc")
                    nc.vector.reciprocal_approx_fast(rc[:], srow[:])
                    rb = rbp.tile([P, TQ], F32, tag="rb")
                    rb2 = rbp.tile([P, TQ], F32, tag="rb2")
                    nc.sync.dma_start(
                        rb[0:DH, :],
                        rc[0:1, 0:TQ].unsqueeze(1)
                          .broadcast_to((1, DH, TQ)))
                    nc.sync.dma_start(
                        rb2[0:DH, :],
                        rc[0:1, TQ:2 * TQ].unsqueeze(1)
                          .broadcast_to((1, DH, TQ)))
                    dst = ot[pl][:, qb * TQ:(qb + 1) * TQ]
                    nc.vector.tensor_mul(dst[0:DH, :], pva[0:DH, :],
                                         rb[0:DH, :])
                    # head b lands at partitions 64-127; DVE cannot shift
                    # partitions: normalize into staging then DMA-shift.
                    sh = rbp.tile([P, TQ], F16, tag="sh")
                    nc.vector.tensor_mul(sh[0:DH, :], pvb[0:DH, :],
                                         rb2[0:DH, :])
                    nc.sync.dma_start(dst[DH:P, :], sh[0:DH, :])
                    # ---- fire the exchange for this (pair, qb) block -----
                    nc.sync.dma_start(ag_in[pl, qb][:], dst)
                    nc.gpsimd.collective_compute(
                        "AllGather", mybir.AluOpType.bypass,
                        replica_groups=RG,
                        ins=[ag_in[pl, qb].opt()],
                        outs=[ag_out[pl, qb].opt()])
                    gather_select(pl, qb)

        # ---------- tail: pair-1 o_proj half + output ----------------
        oproj_half(1)
        assert not filler, f"unconsumed fillers: {list(filler)}"
        if DEBUG_DUMPS:
            for i in range(NPL):
                nc.sync.dma_start(dbg_ot[:, i * T:(i + 1) * T], ot[i][:])
                nc.sync.dma_start(dbg_qt[:, i * T:(i + 1) * T], qt[i][:])
                nc.sync.dma_start(dbg_kt[:, i * T:(i + 1) * T], kt[i][:])
            nc.sync.dma_start(dbg_sel, ot_sel[:])
    nc.compile()
    return nc


_NC_CACHE = None


def _get_nc():
    global _NC_CACHE
    if _NC_CACHE is None:
        _NC_CACHE = build_nc()
    return _NC_CACHE


def _shard_inputs(x, Wqkv, Wo):
    x16 = np.asarray(x, dtype=np.float32).astype(np.float16)
    w16 = np.asarray(Wqkv, dtype=np.float32).astype(np.float16)
    wo16 = np.ascontiguousarray(
        np.asarray(Wo, dtype=np.float32).astype(np.float16))
    in_maps = []
    for c in range(N_CORES):
        b, g = divmod(c, NG)
        cs = slice(g * HPG * DH, (g + 1) * HPG * DH)
        msk = np.zeros((P, NQB), dtype=np.float32)
        msk[:, g] = 1.0
        in_maps.append({
            "xb": np.ascontiguousarray(x16[b]),
            "wq": np.ascontiguousarray(w16[:, 0 * D:1 * D][:, cs]),
            "wk": np.ascontiguousarray(w16[:, 1 * D:2 * D][:, cs]),
            "wv": np.ascontiguousarray(w16[:, 2 * D:3 * D][:, cs]),
            "wo": wo16,
            "msk": msk,
        })
    return in_maps


def kernel(x, Wqkv, Wo):
    from concourse.bass_utils import run_bass_kernel_spmd

    nc = _get_nc()
    in_maps = _shard_inputs(x, Wqkv, Wo)
    res = run_bass_kernel_spmd(nc, in_maps, core_ids=list(range(N_CORES)))
    out = np.empty((B, T, D), dtype=np.float32)
    for c in range(N_CORES):
        b, g = divmod(c, NG)
        out[b, g * TQ:(g + 1) * TQ, :] = res.results[c]["y"]
    return out


# revision 16
# speedup vs baseline: 1.0147x; 1.0147x over previous
"""Multi-head attention (B=2, T=2048, D=1024, H=16, no causal mask) on 8 trn2
NeuronCores — head-parallel sharding with an all-gather exchange before o_proj.

Sharding: core c -> batch b = c//4, head-group g = c%4 (heads [4g, 4g+4), i.e.
local pairs p=0,1 = global pairs 2g+p).  Each core computes Q/K/V projections
and attention for its 4 heads over ALL 2048 queries of its batch (8.6 GFLOP
vs 15.1 for the old data-parallel scheme), then the cores of a batch exchange
attention outputs and each core runs o_proj for its 512-query slice
qs = [g*512, (g+1)*512).

Exchange: 8 small 4-core AllGathers (one per (pair, q-block)) fired as each
(pair, qb) block finishes normalize, so all but the last overlap attention.
AllToAll is unsupported for 4-core groups, so each AG delivers all 4 cores'
[128, 512] blocks and the consumer selects its own q-block with a per-core
one-hot mask input (SPMD-safe: rank enters via input data only) using fused
DVE multiply-adds.

Per-core pipeline (fp16 compute, fp32 PSUM):
  1. PE-transpose X[b] -> XT [128, kd*2048].
  2. QT/KT per pair: [128 (2 heads x 64), 2048]; V slots [128, kc*(4*65)]
     (65-wide: ones column makes PV also produce the softmax denominator).
  3. Attention pair-outer, qb-inner, kc innermost:
       logitsT via two row-tiled (tile_position (0,0)/(64,0)) K=64 matmuls
       that run CONCURRENTLY on the PE -> lg psum [128, 1024]
       exp on ScalarE (the only ACT-engine work in the kernel; ~138 us total,
       near-critical) -> PT f16
       PV accumulate [65, 512] psum over kc.
     normalize: 1/s via reciprocal_approx_fast, DMA partition-broadcast, DVE
     muls; head b partition-shifted via SBUF DMA.
  4. o_proj split: pair-0 slots accumulated into y_acc (SBUF f32) while
     pair-1 attention runs; pair-1 slots added at the tail.
Leftover projection work (V chunks, QT/KT pair 1) is emitted as PE filler
inside the attention loops (the attention inner loop alone is ACT-bound).
"""

import numpy as np

import concourse.bacc as bacc
import concourse.mybir as mybir
import concourse.tile as tile
from concourse.masks import make_identity

F32 = mybir.dt.float32
F16 = mybir.dt.float16

B, T, D, H = 2, 2048, 1024, 16
DH = D // H          # 64
P = 128
KD = D // P          # 8 contraction chunks over D
NT = T // P          # 16 key-token chunks
TQ = 512             # q-block size == o_proj q-slice
NQB = T // TQ        # 4 q blocks
NG = 4               # head-groups (cores per batch)
HPG = H // NG        # 4 heads per core
NPL = HPG // 2       # 2 local pairs
VW = DH + 1          # 65: V head slot width incl. ones column
NQC = TQ // P        # 4 128-row chunks per q-slice
N_CORES = 8
EXPF = mybir.ActivationFunctionType.Exp
MUL = mybir.AluOpType.mult
ADD = mybir.AluOpType.add

RG = [[0, 1, 2, 3], [4, 5, 6, 7]]


DEBUG_DUMPS = False
PRE_PROJ_P1 = False


def build_nc():
    nc = bacc.Bacc("TRN2", target_bir_lowering=False, debug=False,
                   num_devices=N_CORES)
    xb = nc.dram_tensor("xb", [T, D], F16, kind="ExternalInput").ap()
    wq = nc.dram_tensor("wq", [D, HPG * DH], F16, kind="ExternalInput").ap()
    wk = nc.dram_tensor("wk", [D, HPG * DH], F16, kind="ExternalInput").ap()
    wv = nc.dram_tensor("wv", [D, HPG * DH], F16, kind="ExternalInput").ap()
    wo = nc.dram_tensor("wo", [D, D], F16, kind="ExternalInput").ap()
    msk = nc.dram_tensor("msk", [P, NQB], F32, kind="ExternalInput").ap()
    y = nc.dram_tensor("y", [TQ, D], F32, kind="ExternalOutput").ap()
    if DEBUG_DUMPS:
        dbg_ot = nc.dram_tensor("dbg_ot", [P, NPL * T], F16,
                                kind="ExternalOutput").ap()
        dbg_sel = nc.dram_tensor("dbg_sel", [P, H // 2 * TQ], F16,
                                 kind="ExternalOutput").ap()
        dbg_qt = nc.dram_tensor("dbg_qt", [P, NPL * T], F16,
                                kind="ExternalOutput").ap()
        dbg_kt = nc.dram_tensor("dbg_kt", [P, NPL * T], F16,
                                kind="ExternalOutput").ap()

    with tile.TileContext(nc) as tc:
      with (
          tc.tile_pool(name="persist", bufs=1) as persist,
          tc.tile_pool(name="auxps", bufs=2, space="PSUM") as auxps,
          tc.tile_pool(name="xin", bufs=4) as xinp,
          tc.tile_pool(name="ptp", bufs=4) as ptp,
          tc.tile_pool(name="rbp", bufs=4) as rbp,
          tc.tile_pool(name="ginp", bufs=8) as ginp,
          tc.tile_pool(name="dram", bufs=1, space="DRAM") as dram,
      ):
        xt = persist.tile([P, KD * T], F16)          # 32 KB/part
        qt = [persist.tile([P, T], F16, name=f"qt{i}") for i in range(NPL)]
        kt = [persist.tile([P, T], F16, name=f"kt{i}") for i in range(NPL)]
        v_sb = persist.tile([P, NT * HPG * VW], F16)  # 8.1 KB
        ot = [persist.tile([P, T], F16, name=f"ot{i}") for i in range(NPL)]
        ot_sel = [persist.tile([P, NG * TQ], F16, name=f"ot_sel{i}")
                  for i in range(NPL)]      # per-pair gather [r, 512]
        wo_sb = persist.tile([P, (H // 2) * D], F16)  # 16 KB [p, slot, 1024]
        y_acc = persist.tile([P, NQC * D], F32)      # 16 KB
        msk_sb = persist.tile([P, NQB], F32)
        ident = persist.tile([P, P], F16)
        make_identity(nc, ident)
        # ones columns in every (tok-chunk, head) V slot
        onec = persist.tile([P, 1], F16)
        nc.vector.memset(onec[:], 1.0)
        nc.vector.tensor_copy(
            v_sb.rearrange("p (b c) -> p b c", c=VW)[:, :, DH:DH + 1],
            onec.unsqueeze(1).broadcast_to((P, NT * HPG, 1)))

        # weight slices viewed [p, kd, cols] for stationary use
        wq_sb = persist.tile([P, KD * HPG * DH], F16, name="wq_sb")  # 4 KB
        wk_sb = persist.tile([P, KD * HPG * DH], F16, name="wk_sb")
        wv_sb = persist.tile([P, KD * HPG * DH], F16, name="wv_sb")

        def load_small_weights():
            # emitted after the first x chunk so the x DMAs lead the queues
            for w_sb, w in ((wq_sb, wq), (wk_sb, wk), (wv_sb, wv)):
                nc.sync.dma_start(
                    w_sb.rearrange("p (kd c) -> p kd c", c=HPG * DH),
                    w.rearrange("(kd p) c -> p kd c", p=P))
            nc.sync.dma_start(msk_sb[:], msk)

        def load_wo():
            # wo only feeds o_proj; keep its 2 MB strided DMA off the
            # critical input path
            nc.sync.dma_start(
                wo_sb.rearrange("p (s c) -> p s c", c=D),
                wo.rearrange("(s p) c -> p s c", p=P))

        ag_in = {}
        ag_out = {}
        for pl in range(NPL):
            for qb in range(NQB):
                ag_in[pl, qb] = dram.tile([P, TQ], F16,
                                          name=f"agin_{pl}_{qb}")
                ag_out[pl, qb] = dram.tile([NG, P, TQ], F16,
                                           name=f"agout_{pl}_{qb}")

        # ---------- helpers ------------------------------------------
        def proj_qk_block(w_sb, dst, pl, qb):
            # dst[pl][:, qb*TQ :] = W[:, pair pl]^T @ XT[:, qb block]
            pq = auxps.tile([P, TQ], F32, tag="aux")
            for kd in range(KD):
                nc.tensor.matmul(
                    pq[:],
                    w_sb[:, kd * HPG * DH + pl * P:
                         kd * HPG * DH + (pl + 1) * P],
                    xt[:, kd * T + qb * TQ: kd * T + (qb + 1) * TQ],
                    start=(kd == 0), stop=(kd == KD - 1))
            nc.vector.tensor_copy(
                dst[pl][:, qb * TQ: (qb + 1) * TQ], pq[:])

        def proj_v_chunk(tci):
            # V rows for tokens [tci*128, ..): [128, 256] -> 65-wide slots
            pv = auxps.tile([P, TQ], F32, tag="aux")
            for kd in range(KD):
                nc.tensor.matmul(
                    pv[:, 0:HPG * DH],
                    xt[:, kd * T + tci * P: kd * T + (tci + 1) * P],
                    wv_sb[:, kd * HPG * DH:(kd + 1) * HPG * DH],
                    start=(kd == 0), stop=(kd == KD - 1))
            dst = v_sb[:, tci * (HPG * VW): (tci + 1) * (HPG * VW)]
            nc.vector.tensor_copy(
                dst.rearrange("p (h c) -> p h c", c=VW)[:, :, 0:DH],
                pv[:, 0:HPG * DH].rearrange("p (h c) -> p h c", c=DH))

        # select-accumulate AG output block into ot_sel slots.  These ops
        # depend on the AllGather; they are EMITTED ~2 attention blocks
        # after the AG fires, so by the time they reach the head of the
        # DVE/gpsimd FIFOs the data has landed and nothing stalls.
        def gather_select(pl, qb):
            for r in range(NG):
                gin = ginp.tile([P, TQ], F16, tag="gin")
                nc.gpsimd.dma_start(gin[:], ag_out[pl, qb][r])
                dst = ot_sel[pl][:, r * TQ:(r + 1) * TQ]
                if qb == 0:
                    nc.vector.tensor_scalar_mul(
                        dst, gin[:], msk_sb[:, qb:qb + 1])
                else:
                    nc.vector.scalar_tensor_tensor(
                        dst, gin[:], msk_sb[:, qb:qb + 1], dst,
                        op0=MUL, op1=ADD)

        def oproj_half(pl):
            # accumulate this pair-half's 4 slots into y_acc / emit y
            for qc in range(NQC):
                for nh in range(2):
                    py = auxps.tile([P, TQ], F32, tag="aux")
                    for r in range(NG):
                        slot = 2 * r + pl
                        nc.tensor.matmul(
                            py[:],
                            ot_sel[pl][:, r * TQ + qc * P:
                                       r * TQ + (qc + 1) * P],
                            wo_sb[:, slot * D + nh * TQ:
                                  slot * D + (nh + 1) * TQ],
                            start=(r == 0), stop=(r == NG - 1))
                    ya = y_acc[:, qc * D + nh * TQ: qc * D + (nh + 1) * TQ]
                    if pl == 0:
                        nc.vector.tensor_copy(ya, py[:])
                    else:
                        yo = rbp.tile([P, TQ], F32, tag="yout", bufs=3)
                        nc.vector.tensor_add(yo[:], py[:], ya)
                        nc.sync.dma_start(
                            y[qc * P:(qc + 1) * P, nh * TQ:(nh + 1) * TQ],
                            yo[:])

        # ---------- pre-region ---------------------------------------
        with tc.tile_pool(name="trps", bufs=3, space="PSUM") as trps:
            for tci in range(NT):
                xin = xinp.tile([P, D], F16, tag="xin")
                nc.sync.dma_start(xin[:], xb[tci * P:(tci + 1) * P, :])
                if tci == 0:
                    load_small_weights()
                if tci == NT - 1:
                    load_wo()
                ps = trps.tile([P, KD * P], F16, tag="tr")
                for kd in range(KD):
                    nc.tensor.transpose(
                        ps[:, kd * P:(kd + 1) * P],
                        xin[:, kd * P:(kd + 1) * P], ident[:])
                nc.vector.tensor_copy(
                    xt.rearrange("p (k c) -> p k c", c=T)
                      [:, :, tci * P:(tci + 1) * P],
                    ps.rearrange("p (k c) -> p k c", c=P))

        for qb in range(NQB):
            proj_qk_block(wk_sb, kt, 0, qb)
        proj_qk_block(wq_sb, qt, 0, 0)
        for tci in range(2):
            proj_v_chunk(tci)

        # filler work emitted inside the attention loops, keyed by
        # (pair, qb, kc) slot.  Each item is a closure.
        filler = {}

        def add_filler(pl, qb, kc, fn):
            filler.setdefault((pl, qb, kc), []).append(fn)

        # V chunks tci=2..15 inside pair0 qb0: PV of kc reads v chunk kc,
        # so chunk tci must be emitted before PV tci — keep 2 ahead.
        for tci in range(2, NT):
            add_filler(0, 0, tci - 2, lambda t=tci: proj_v_chunk(t))
        # QT pair0 qb+1 emitted mid-way through qb
        for qb in range(NQB - 1):
            add_filler(0, qb, 10, lambda q=qb + 1:
                       proj_qk_block(wq_sb, qt, 0, q))
        # KT/QT pair1 during pair0 qb2/qb3
        for qb in range(NQB):
            add_filler(0, 2, 1 + 4 * qb, lambda q=qb:
                       proj_qk_block(wk_sb, kt, 1, q))
            add_filler(0, 3, 1 + 4 * qb, lambda q=qb:
                       proj_qk_block(wq_sb, qt, 1, q))
        if PRE_PROJ_P1:
            # debug/workaround: run pair-1 projections sequentially in the
            # pre-region instead of interleaved with attention.
            for key in [k for k in filler if k[0] == 0 and k[1] >= 2]:
                for fn in filler.pop(key):
                    fn()
        # deferred gather-selects: each ~2 blocks after its AG fires
        SELECT_SLOTS = {(0, 0): (0, 2, 12), (0, 1): (0, 3, 12),
                        (0, 2): (1, 0, 12), (0, 3): (1, 1, 4),
                        (1, 0): (1, 2, 4), (1, 1): (1, 2, 12),
                        (1, 2): (1, 3, 12)}
        for (spl, sqb), where in SELECT_SLOTS.items():
            add_filler(*where, lambda a=spl, b=sqb: gather_select(a, b))
        # pair0 gathers have landed by mid-pair1; fold their o_proj
        # half in as PE filler.
        add_filler(1, 1, 8, lambda: oproj_half(0))

        # ---------- attention ----------------------------------------
        with (
            tc.tile_pool(name="lgps", bufs=2, space="PSUM") as lgps,
            tc.tile_pool(name="pvps", bufs=2, space="PSUM") as pvps,
        ):
            for pl in range(NPL):
                for qb in range(NQB):
                    pva = pvps.tile([VW, TQ], F32, tag="pv")
                    pvb = pvps.tile([VW, TQ], F32, tag="pv")
                    for kc in range(NT):
                        for fn in filler.pop((pl, qb, kc), ()):
                            fn()
                        lg = lgps.tile([P, 2 * TQ], F32, tag="lg")
                        for hh in range(2):
                            nc.tensor.matmul(
                                lg[:, hh * TQ:(hh + 1) * TQ],
                                kt[pl][hh * DH:(hh + 1) * DH,
                                       kc * P:(kc + 1) * P],
                                qt[pl][hh * DH:(hh + 1) * DH,
                                       qb * TQ:(qb + 1) * TQ],
                                start=True, stop=True,
                                tile_position=(hh * DH, 0))
                        pt = ptp.tile([P, 2 * TQ], F16, tag="pt")
                        nc.scalar.activation(pt[:], lg[:], EXPF, scale=0.125)
                        for hh, pv_ in ((0, pva), (1, pvb)):
                            h = 2 * pl + hh
                            va = v_sb[:, kc * (HPG * VW) + h * VW:
                                      kc * (HPG * VW) + (h + 1) * VW]
                            nc.tensor.matmul(
                                pv_[:], va, pt[:, hh * TQ:(hh + 1) * TQ],
                                start=(kc == 0), stop=(kc == NT - 1))
                    # ---- normalize: ot[:, blk] = pv[0:64] / s ------------
                    srow = rbp.tile([1, 2 * TQ], F32, tag="srow")
                    nc.vector.tensor_copy(srow[0:1, 0:TQ],
                                          pva[DH:DH + 1, :])
                    nc.vector.tensor_copy(srow[0:1, TQ:2 * TQ],
                                          pvb[DH:DH + 1, :])
                    rc = rbp.tile([1, 2 * TQ], F32, tag="rc")
                    nc.vector.reciprocal_approx_fast(rc[:], srow[:])
                    rb = rbp.tile([P, TQ], F32, tag="rb")
                    rb2 = rbp.tile([P, TQ], F32, tag="rb2")
                    nc.sync.dma_start(
                        rb[0:DH, :],
                        rc[0:1, 0:TQ].unsqueeze(1)
                          .broadcast_to((1, DH, TQ)))
                    nc.sync.dma_start(
                        rb2[0:DH, :],
                        rc[0:1, TQ:2 * TQ].unsqueeze(1)
                          .broadcast_to((1, DH, TQ)))
                    dst = ot[pl][:, qb * TQ:(qb + 1) * TQ]
                    nc.vector.tensor_mul(dst[0:DH, :], pva[0:DH, :],
                                         rb[0:DH, :])
                    # head b lands at partitions 64-127; DVE cannot shift
                    # partitions: normalize into staging then DMA-shift.
                    sh = rbp.tile([P, TQ], F16, tag="sh")
                    nc.vector.tensor_mul(sh[0:DH, :], pvb[0:DH, :],
                                         rb2[0:DH, :])
                    nc.sync.dma_start(dst[DH:P, :], sh[0:DH, :])
                    # ---- fire the exchange for this (pair, qb) block -----
                    nc.sync.dma_start(ag_in[pl, qb][:], dst)
                    nc.gpsimd.collective_compute(
                        "AllGather", mybir.AluOpType.bypass,
                        replica_groups=RG,
                        ins=[ag_in[pl, qb].opt()],
                        outs=[ag_out[pl, qb].opt()])

        # ---------- tail: last select + pair-1 o_proj half ------------
        gather_select(1, 3)
        oproj_half(1)
        assert not filler, f"unconsumed fillers: {list(filler)}"
        if DEBUG_DUMPS:
            for i in range(NPL):
                nc.sync.dma_start(dbg_ot[:, i * T:(i + 1) * T], ot[i][:])
                nc.sync.dma_start(dbg_qt[:, i * T:(i + 1) * T], qt[i][:])
                nc.sync.dma_start(dbg_kt[:, i * T:(i + 1) * T], kt[i][:])
                nc.sync.dma_start(dbg_sel[:, i * NG * TQ:(i + 1) * NG * TQ],
                                  ot_sel[i][:])
    nc.compile()
    return nc


_NC_CACHE = None


def _get_nc():
    global _NC_CACHE
    if _NC_CACHE is None:
        _NC_CACHE = build_nc()
    return _NC_CACHE


def _shard_inputs(x, Wqkv, Wo):
    x16 = np.asarray(x, dtype=np.float32).astype(np.float16)
    w16 = np.asarray(Wqkv, dtype=np.float32).astype(np.float16)
    wo16 = np.ascontiguousarray(
        np.asarray(Wo, dtype=np.float32).astype(np.float16))
    in_maps = []
    for c in range(N_CORES):
        b, g = divmod(c, NG)
        cs = slice(g * HPG * DH, (g + 1) * HPG * DH)
        msk = np.zeros((P, NQB), dtype=np.float32)
        msk[:, g] = 1.0
        in_maps.append({
            "xb": np.ascontiguousarray(x16[b]),
            "wq": np.ascontiguousarray(w16[:, 0 * D:1 * D][:, cs]),
            "wk": np.ascontiguousarray(w16[:, 1 * D:2 * D][:, cs]),
            "wv": np.ascontiguousarray(w16[:, 2 * D:3 * D][:, cs]),
            "wo": wo16,
            "msk": msk,
        })
    return in_maps


def kernel(x, Wqkv, Wo):
    from concourse.bass_utils import run_bass_kernel_spmd

    nc = _get_nc()
    in_maps = _shard_inputs(x, Wqkv, Wo)
    res = run_bass_kernel_spmd(nc, in_maps, core_ids=list(range(N_CORES)))
    out = np.empty((B, T, D), dtype=np.float32)
    for c in range(N_CORES):
        b, g = divmod(c, NG)
        out[b, g * TQ:(g + 1) * TQ, :] = res.results[c]["y"]
    return out


# revision 17
# speedup vs baseline: 1.1471x; 1.1304x over previous
"""Multi-head attention (B=2, T=2048, D=1024, H=16, no causal mask) on 8 trn2
NeuronCores.

Sharding: pure data-parallel over (batch, query-token-block).  Core c handles
batch b = c // 4 and query rows [tb*512, (tb+1)*512) with tb = c % 4.  Each
core redundantly computes K and V for its whole batch (15.1 GFLOP/core vs 8.6
for tensor-parallel-heads) but needs NO collectives; an on-chip 4-rank 8 MB
AllReduce costs more than the redundant compute.

Precision: x/Wqkv and the whole QKV+attention pipeline run in fp16 (PE at
1 cyc/row with fp32 PSUM accumulation; fp32r measures ~2 cyc/row on HW even
warm; fp16 keeps 3 more mantissa bits than bf16 -> ~6e-4 final rel err vs
5e-3).  The output projection — whose input rounding feeds the result
directly — stays in float32r.

Per-core plan:
  1. PE-transpose X[b] (bf16) into XT (1024x2048 SBUF) and the query slice
     Xq into XqT.
  2. QT[do,:] = Wq[:,do]^T @ XqT       (QT:  [1024, 512]  bf16 SBUF)
     KT[do,:] = Wk[:,do]^T @ XT        (KT:  [1024, 2048] bf16 SBUF)
     V [tc,:] = XT[:,tc]^T @ Wv        (V:   [2048, 1024] bf16 SBUF, 65-wide
                                        head slots with a ones column ->
                                        PV also yields the softmax sum)
  3. Attention per head pair p (heads 2p, 2p+1 at partitions 0-63 / 64-127
     of KT/QT row chunk p):
       logitsT[k,q] = KT_h[:,kc]^T @ QT_h      (PSUM [128, 512] fp32)
       PT = exp(0.125 * logitsT)               (ScalarE, PSUM -> bf16 SBUF)
       outT_h[dh,q], s[q] = [V_h | 1]^T @ PT   (PSUM [65, 512], accum 16 kc)
     normalize: outT_h *= (1/s) broadcast across partitions via DMA,
     written to ot_sb as float32r.
  4. y[q,:] = outT^T @ Wo in float32r (accumulate 8 row chunks).
"""

import numpy as np

import concourse.bacc as bacc
import concourse.mybir as mybir
import concourse.tile as tile
from concourse.masks import make_identity

F32 = mybir.dt.float32
F32R = mybir.dt.float32r
F16 = mybir.dt.float16

B, T, D, H = 2, 2048, 1024, 16
DH = D // H  # 64
TQ = 512     # query tokens per core
N_CORES = 8
P = 128
KD = D // P        # 8 contraction chunks over D
NT = T // P        # 16 key-token chunks
NTB = T // TQ      # 4 token blocks
NPAIR = H // 2     # 8 head pairs
VW = DH + 1        # 65: V head slot width incl. ones column
NQ = TQ // P       # 4 query-token chunks
EXPF = mybir.ActivationFunctionType.Exp


def build_nc():
    nc = bacc.Bacc("TRN2", target_bir_lowering=False, debug=False,
                   num_devices=N_CORES)
    xb = nc.dram_tensor("xb", [T, D], F16, kind="ExternalInput").ap()
    xq = nc.dram_tensor("xq", [TQ, D], F16, kind="ExternalInput").ap()
    wqkv = nc.dram_tensor("wqkv", [D, 3 * D], F16, kind="ExternalInput").ap()
    wo = nc.dram_tensor("wo", [D, D], F16, kind="ExternalInput").ap()
    y = nc.dram_tensor("y", [TQ, D], F32, kind="ExternalOutput").ap()

    with tile.TileContext(nc) as tc:
      with tc.tile_pool(name="persist", bufs=1) as persist:
        v_sb = persist.tile([P, NT * H * VW], F16)    # 32.5 KB/part
        qt_sb = persist.tile([P, NPAIR * TQ], F16)    # 8 KB/part
        kt_sb = persist.tile([P, KD * T], F16)        # 32 KB/part
        ident = persist.tile([P, P], F16)
        make_identity(nc, ident)
        # ones columns in every (tok-chunk, head) V slot
        onec = persist.tile([P, 1], F16)
        nc.vector.memset(onec[:], 1.0)
        nc.vector.tensor_copy(
            v_sb.rearrange("p (b c) -> p b c", c=VW)[:, :, DH:DH + 1],
            onec.unsqueeze(1).broadcast_to((P, NT * H, 1)))

        # wqkv viewed as [p, ko, col]: one DMA per weight column strip
        wq3 = wqkv.rearrange("(ko p) c -> p ko c", p=P)

        with (
            tc.tile_pool(name="xtp", bufs=1) as xtp,   # spans proj + attention
            tc.tile_pool(name="wp", bufs=1) as wp,
        ):
            xt = xtp.tile([P, KD * T], F16)    # 32 KB/part
            xqt = xtp.tile([P, KD * TQ], F16)  # 8 KB/part

            # ---------- pre-region: A (transpose), C (V), D (QT), B[do=0] ---
            with (
                tc.tile_pool(name="xin", bufs=3) as xinp,
                tc.tile_pool(name="trps", bufs=3, space="PSUM") as trps,
                tc.tile_pool(name="pjps", bufs=5, space="PSUM") as pjps,
            ):
                # A: PE-transpose xq then xb.  8 kd-subtiles share one psum
                # bank; one strided copy scatters them into xt/xqt.
                def transpose_chunk(src_row, dst, dst_off, dst_stride):
                    ps = trps.tile([P, KD * P], F16, tag="tr")
                    for kd in range(KD):
                        nc.tensor.transpose(
                            ps[:, kd * P:(kd + 1) * P],
                            src_row[:, kd * P:(kd + 1) * P], ident[:])
                    nc.vector.tensor_copy(
                        dst.rearrange("p (k c) -> p k c", c=dst_stride)
                           [:, :, dst_off:dst_off + P],
                        ps.rearrange("p (k c) -> p k c", c=P))

                for tci in range(NQ):
                    xin = xinp.tile([P, D], F16, tag="xin")
                    nc.sync.dma_start(xin[:], xq[tci * P:(tci + 1) * P, :])
                    transpose_chunk(xin, xqt, tci * P, TQ)
                for tci in range(NT):
                    xin = xinp.tile([P, D], F16, tag="xin")
                    nc.sync.dma_start(xin[:], xb[tci * P:(tci + 1) * P, :])
                    transpose_chunk(xin, xt, tci * P, T)

                # D: QT (dout chunk do covers heads 2do, 2do+1)
                for do in range(KD):
                    wt = wp.tile([P, KD * P], F16, tag="wk", bufs=2)
                    nc.sync.dma_start(
                        wt.rearrange("p (ko c) -> p ko c", c=P),
                        wq3[:, :, do * P:(do + 1) * P])
                    pq = pjps.tile([P, TQ], F32, tag="pj")
                    for kd in range(KD):
                        nc.tensor.matmul(
                            pq[:], wt[:, kd * P:(kd + 1) * P],
                            xqt[:, kd * TQ:(kd + 1) * TQ],
                            start=(kd == 0), stop=(kd == KD - 1))
                    nc.vector.tensor_copy(qt_sb[:, do * TQ:(do + 1) * TQ], pq[:])

                # C: V natural, into 65-wide head slots
                for nh in range(2):
                    wvt = wp.tile([P, KD * TQ], F16, tag="wv", bufs=2)
                    nc.sync.dma_start(
                        wvt.rearrange("p (ko c) -> p ko c", c=TQ),
                        wq3[:, :, 2 * D + nh * TQ: 2 * D + (nh + 1) * TQ])
                    for tci in range(NT):
                        pv = pjps.tile([P, TQ], F32, tag="pj")
                        for kd in range(KD):
                            nc.tensor.matmul(
                                pv[:],
                                xt[:, kd * T + tci * P: kd * T + (tci + 1) * P],
                                wvt[:, kd * TQ:(kd + 1) * TQ],
                                start=(kd == 0), stop=(kd == KD - 1))
                        dst = v_sb[:, tci * (H * VW) + nh * 8 * VW:
                                   tci * (H * VW) + (nh + 1) * 8 * VW]
                        nc.vector.tensor_copy(
                            dst.rearrange("p (h c) -> p h c", c=VW)[:, :, 0:DH],
                            pv.rearrange("p (h c) -> p h c", c=DH))

                # B[do=0]: pre-computed so pair 0 can start immediately;
                # kd-outer with 4 open accumulators amortizes LDWEIGHTS.
                wt = wp.tile([P, KD * P], F16, tag="wk", bufs=2)
                nc.sync.dma_start(
                    wt.rearrange("p (ko c) -> p ko c", c=P),
                    wq3[:, :, D: D + P])
                pks = [pjps.tile([P, TQ], F32, tag="pj", name=f"pk0_{_t}")
                       for _t in range(NTB)]
                for kd in range(KD):
                    for tb in range(NTB):
                        nc.tensor.matmul(
                            pks[tb][:], wt[:, kd * P:(kd + 1) * P],
                            xt[:, kd * T + tb * TQ: kd * T + (tb + 1) * TQ],
                            start=(kd == 0), stop=(kd == KD - 1))
                for tb in range(NTB):
                    nc.vector.tensor_copy(
                        kt_sb[:, tb * TQ:(tb + 1) * TQ], pks[tb][:])

            # ---------- region: attention pairs interleaved with B[do] -----
            with (
                tc.tile_pool(name="otp", bufs=1) as otp,
                tc.tile_pool(name="ptp", bufs=4) as ptp,
                tc.tile_pool(name="rcp", bufs=2) as rcp,
                tc.tile_pool(name="rbp", bufs=3) as rbp,
                tc.tile_pool(name="wop", bufs=16) as wop,
            ):
                ot_sb = otp.tile([P, NPAIR * TQ], F16)      # 8 KB/part
                # preload Wo so phase F never waits on DMA
                wot = {}
                for ph in range(NPAIR):
                    for nh in range(2):
                        wot[ph, nh] = wop.tile([P, TQ], F16, tag="wo",
                                               name=f"wo_{ph}_{nh}")
                        nc.sync.dma_start(
                            wot[ph, nh][:],
                            wo[ph * P:(ph + 1) * P, nh * TQ:(nh + 1) * TQ])

                GRP = 2   # 512-wide logits halves per psum tile / exp call
                attn_ps = tc.tile_pool(name="lgps", bufs=2, space="PSUM")
                lgps = attn_ps.__enter__()
                pv_ps = tc.tile_pool(name="pvps", bufs=2, space="PSUM")
                pvps = pv_ps.__enter__()
                b_ps = tc.tile_pool(name="bjps", bufs=2, space="PSUM")
                bjps = b_ps.__enter__()

                def emit_b_chunk(do, tb):
                    # One KT (row chunk do, token block tb) accumulation:
                    # PE filler spread through the ACT-bound attention.
                    wt = bwt[do]
                    pk = bjps.tile([P, TQ], F32, tag="bk",
                                   name=f"pkb{do}_{tb}")
                    for kd in range(KD):
                        nc.tensor.matmul(
                            pk[:], wt[:, kd * P:(kd + 1) * P],
                            xt[:, kd * T + tb * TQ: kd * T + (tb + 1) * TQ],
                            start=(kd == 0), stop=(kd == KD - 1))
                    nc.vector.tensor_copy(
                        kt_sb[:, do * T + tb * TQ: do * T + (tb + 1) * TQ],
                        pk[:])

                bwt = {}

                def fetch_b_weights(do):
                    bwt[do] = wp.tile([P, KD * P], F16, tag="wk", bufs=2,
                                      name=f"wtb{do}")
                    nc.sync.dma_start(
                        bwt[do].rearrange("p (ko c) -> p ko c", c=P),
                        wq3[:, :, D + do * P: D + (do + 1) * P])

                for p in range(NPAIR):
                    if p + 1 < NPAIR:
                        fetch_b_weights(p + 1)
                    kt = kt_sb[:, p * T:(p + 1) * T]
                    qh = (qt_sb[0:DH, p * TQ:(p + 1) * TQ],
                          qt_sb[DH:P, p * TQ:(p + 1) * TQ])
                    pva = pvps.tile([VW, TQ], F32, tag="pv")
                    pvb = pvps.tile([VW, TQ], F32, tag="pv")
                    halves = [(kc, hh) for kc in range(NT) for hh in (0, 1)]
                    groups = [halves[i:i + GRP]
                              for i in range(0, len(halves), GRP)]
                    loc = {}
                    emitted = set()

                    def emit_pv_ready(done_through):
                        for kc in range(NT):
                            if kc in emitted:
                                continue
                            if ((kc, 0) not in done_through
                                    or (kc, 1) not in done_through):
                                return
                            emitted.add(kc)
                            for hh, pv_ in ((0, pva), (1, pvb)):
                                h = 2 * p + hh
                                va = v_sb[:, kc * (H * VW) + h * VW:
                                          kc * (H * VW) + h * VW + VW]
                                pt_, j = loc[kc, hh]
                                nc.tensor.matmul(pv_[:], va,
                                                 pt_[:, j * TQ:(j + 1) * TQ],
                                                 start=(kc == 0),
                                                 stop=(kc == NT - 1))

                    done = set()
                    prev_done = set()
                    for gi, grp in enumerate(groups):
                        # spread next pair's KT chunks through this pair
                        if p + 1 < NPAIR and gi % 4 == 2:
                            emit_b_chunk(p + 1, gi // 4)
                        emit_pv_ready(prev_done)
                        n = len(grp)
                        lg = lgps.tile([P, GRP * TQ], F32, tag="lg")
                        for j, (kc, hh) in enumerate(grp):
                            nc.tensor.matmul(
                                lg[:, j * TQ:(j + 1) * TQ],
                                kt[hh * DH:(hh + 1) * DH,
                                   kc * P:(kc + 1) * P],
                                qh[hh], start=True, stop=True)
                        pt_ = ptp.tile([P, GRP * TQ], F16, tag="pt")
                        nc.scalar.activation(pt_[:, 0:n * TQ],
                                             lg[:, 0:n * TQ],
                                             EXPF, scale=0.125)
                        for j, half in enumerate(grp):
                            loc[half] = (pt_, j)
                        prev_done = set(done)
                        done.update(grp)
                    emit_pv_ready(done)

                    # normalize: outT_h[dh, q] *= 1 / s[q].  Copy psum out
                    # first so the PV banks free fast for the next pair.
                    for hi, pv_ in ((0, pva), (1, pvb)):
                        pvs = rcp.tile([VW, TQ], F32, tag="pvs")
                        nc.vector.tensor_copy(pvs[:], pv_[:])
                        # 1/s on partition 0 via the fast approx custom op
                        # (plain reciprocal costs ~3.3us per call; the
                        # approx op mishandles base partition 64, so copy
                        # the sum row down to partition 0 first)
                        sr = rcp.tile([1, TQ], F32, tag="sr")
                        nc.vector.tensor_copy(sr[0:1, :],
                                              pvs[DH:DH + 1, :])
                        rc = rcp.tile([1, TQ], F32, tag="rc")
                        nc.vector.reciprocal_approx_fast(rc[0:1, :],
                                                         sr[0:1, :])
                        rb = rbp.tile([P, TQ], F32, tag="rb")
                        nc.sync.dma_start(
                            rb[0:DH, :],
                            rc[0:1, :].unsqueeze(1)
                              .broadcast_to((1, DH, TQ)))
                        if hi == 0:
                            nc.vector.tensor_mul(
                                ot_sb[0:DH, p * TQ:(p + 1) * TQ],
                                pvs[0:DH, :], rb[0:DH, :])
                        else:
                            # head b lands at partitions 64-127 of ot_sb;
                            # DVE cannot shift partitions, so normalize into
                            # a staging tile then DMA-shift.
                            sh = rbp.tile([P, TQ], F16, tag="sh")
                            nc.vector.tensor_mul(
                                sh[0:DH, :], pvs[0:DH, :], rb[0:DH, :])
                            nc.sync.dma_start(
                                ot_sb[DH:P, p * TQ:(p + 1) * TQ],
                                sh[0:DH, :])
                b_ps.__exit__(None, None, None)
                pv_ps.__exit__(None, None, None)
                attn_ps.__exit__(None, None, None)

                # F: y = outT^T @ Wo (ph-outer reuses each stationary twice)
                with tc.tile_pool(name="fps", bufs=4, space="PSUM") as fps:
                  for qc in range(NQ):
                    pys = [fps.tile([P, TQ], F32, tag="f", name=f"py{qc}_{_n}")
                           for _n in range(2)]
                    for ph in range(NPAIR):
                        for nh in range(2):
                            nc.tensor.matmul(
                                pys[nh][:],
                                ot_sb[:, ph * TQ + qc * P:
                                      ph * TQ + (qc + 1) * P],
                                wot[ph, nh][:],
                                start=(ph == 0), stop=(ph == NPAIR - 1))
                    for nh in range(2):
                        ys = rbp.tile([P, TQ], F32, tag="rb")
                        nc.vector.tensor_copy(ys[:], pys[nh][:])
                        nc.sync.dma_start(
                            y[qc * P:(qc + 1) * P, nh * TQ:(nh + 1) * TQ],
                            ys[:])
    nc.compile()
    return nc


_NC_CACHE = None


def _get_nc():
    global _NC_CACHE
    if _NC_CACHE is None:
        _NC_CACHE = build_nc()
    return _NC_CACHE


def _shard_inputs(x, Wqkv, Wo):
    x16 = np.asarray(x, dtype=np.float32).astype(np.float16)
    w16 = np.ascontiguousarray(np.asarray(Wqkv, dtype=np.float32).astype(np.float16))
    wo16 = np.ascontiguousarray(np.asarray(Wo, dtype=np.float32).astype(np.float16))
    in_maps = []
    for c in range(N_CORES):
        b, tb = c // NTB, c % NTB
        in_maps.append({
            "xb": np.ascontiguousarray(x16[b]),
            "xq": np.ascontiguousarray(x16[b, tb * TQ:(tb + 1) * TQ, :]),
            "wqkv": w16,
            "wo": wo16,
        })
    return in_maps


def kernel(x, Wqkv, Wo):
    from concourse.bass_utils import run_bass_kernel_spmd

    nc = _get_nc()
    in_maps = _shard_inputs(x, Wqkv, Wo)
    res = run_bass_kernel_spmd(nc, in_maps, core_ids=list(range(N_CORES)))
    out = np.empty((B, T, D), dtype=np.float32)
    for c in range(N_CORES):
        b, tb = c // NTB, c % NTB
        out[b, tb * TQ:(tb + 1) * TQ, :] = res.results[c]["y"]
    return out

